# revision 36
# baseline (speedup 1.0000x reference)
"""Trainium2 Bass kernel for nn_CELoss_4896262717859 (fp8 DoubleRow, gathered
query columns).

For each query column c = idx_node[k] of a sparse adjacency matrix (diagonal
zeroed), a cross-entropy-style loss over the "lower" (r < c) and "upper"
(r > c) neighbor sets:

    contrib_side(c) = [cnt>0 and poscnt==1] * (log(sum_r m exp(out_r)) - poslogit) / cnt

All per-column quantities are sums  sum_r adj[r,c] * w[r]  for
w in {1, pos, pos*out, exp(out)} -> tensor-engine matvecs with a triangular
(L/U) split, computed ONLY for the distinct idx_node columns (~3218 of 8192),
then combined with multiplicities on the host (O(N+K)).

Sharding: core d handles the distinct query columns falling in column slab
[1024d, 1024(d+1)).  Within a slab, columns are bucketed by the 128-row block
containing their diagonal (the "mixed" block); each of the 8 buckets is padded
to a fixed BCAP=56 slots -> exactly 448 column slots per core, so ONE compiled
program (fixed matmul ranges) serves every core and any input.  Rows are
rotated by 1024d so the mixed blocks always land in local row-tiles 0..7.
Bucket overflow beyond 56 distinct columns (a handful for uniform idx_node)
falls back to an exact host-side computation for the overflowed columns only.

Everything streams as fp8e4 (adjacency 0/1 exact; weights hi/mid/lo split
-> ~12 mantissa bits): 3.7 MB/core vs the 32 MB int32 baseline.  Row-tile
pairs run as DoubleRow matmuls (2 fp8 MACs/cell/cycle), halving PE time; the
diagonal tiles pair their shared L/U spans the same way.  The mixed 128-row
block of each column is pre-masked on the host: its lower part (rows < c)
replaces the block in the main slab (covered by the L matmul), its upper part
goes to a small separate diagu[128,448] operand (one extra 56-wide matmul per
diagonal tile).  No on-device casts or mask multiplies remain.

DMA: the adjacency chunks stream through the single gpsimd SWDGE ring in
need order (per-chunk contiguous dram tensors; small leading chunks so
matmuls start early, small final chunk so the compute tail after the last
arrival is short); weights/diagu load concurrently via the sync/scalar HWDGE
rings.  Concurrent chunk rings measured strictly worse (fair-share packet
round-robin delays in-order arrival); per-core streaming tops out ~280 GB/s
regardless of ring mix, making the stream the critical path.  The psum bank
is copied out in halves (ACT+DVE) and written back through both HWDGE rings.
"""

import numpy as np
import ml_dtypes

N = 8192
K = 4096
NCORES = 8
SLAB = N // NCORES        # 1024 columns per slab
P = 128                   # partition / tile edge
NT = N // P               # 64 row tiles
TPC = SLAB // P           # 8 diagonal (mixed) tiles per core
NW = 8                    # weights per side: {1, pos, pl_h, pl_m, pl_l, e_h, e_m, e_l}
M = 2 * NW                # 16 psum partitions (L half = 0:8, U half = 8:16)
VW = 16                   # weight-variant stride (cols); == M, and 16B for fp8
CAP = 448                 # column slots per core (one psum bank)
BCAP = CAP // TPC         # 56 slots per 128-row bucket
CHUNK_TILES = (2, 2, 4, 40, 12, 4)   # row tiles per DMA chunk
CHUNK_ENGS = ("gpsimd",) * 6         # issuing ring per chunk

BF16 = ml_dtypes.bfloat16
FP8 = ml_dtypes.float8_e4m3fn

_BASS_CACHE = {}


def _build_bass():
    import concourse.tile as tile
    import concourse.mybir as mybir
    from concourse import bacc

    f8 = mybir.dt.float8e4
    DR = mybir.MatmulPerfMode.DoubleRow
    # Bacc (not raw Bass): its compile() runs generate_event_semaphores,
    # which splits multi-sem waits — TRN2 instructions hold at most one.
    nc = bacc.Bacc("TRN2")
    adjs = [
        nc.dram_tensor(f"adj{ci}", [P, nt * CAP], f8, kind="ExternalInput")
        for ci, nt in enumerate(CHUNK_TILES)
    ]
    wmat = nc.dram_tensor("wmat", [P, (NT + TPC) * VW], f8, kind="ExternalInput")
    diagu = nc.dram_tensor("diagu", [P, CAP], f8, kind="ExternalInput")
    stats = nc.dram_tensor("stats", [M, CAP], mybir.dt.float32, kind="ExternalOutput")

    with tile.TileContext(nc) as tc:
        with (
            tc.tile_pool(name="singles", bufs=1) as singles,
            tc.tile_pool(name="psum", bufs=1, space="PSUM") as psum_pool,
        ):
            # weights/diagu on the two HWDGE rings (needed first; they drain
            # while the chunk stream starts), adjacency chunks on the gpsimd
            # SWDGE ring as one in-order FIFO.
            asb = singles.tile([P, NT + TPC, VW], f8)
            nc.sync.dma_start(out=asb, in_=wmat[:, :])
            dsb = singles.tile([P, CAP], f8)
            nc.scalar.dma_start(out=dsb, in_=diagu[:, :])
            engs = {"sync": nc.sync, "scalar": nc.scalar, "gpsimd": nc.gpsimd}
            chunks = []   # (tile, first_tile, ntiles)
            t0 = 0
            for ci, nt in enumerate(CHUNK_TILES):
                t = singles.tile([P, nt, CAP], f8, name=f"ch{ci}")
                engs[CHUNK_ENGS[ci]].dma_start(out=t, in_=adjs[ci][:, :])
                chunks.append((t, t0, nt))
                t0 += nt
            assert t0 == NT

            acc = psum_pool.tile([M, CAP], mybir.dt.float32, name="acc")

            def wv(v, n=1):
                return asb[:, v : v + n, :]

            def du(b):  # diagu strip for mixed block b
                return dsb[:, BCAP * b : BCAP * (b + 1)]

            def chunk_rhs(j, n=1):
                for t, t0, nt in chunks:
                    if t0 <= j and j + n <= t0 + nt:
                        return t[:, j - t0 : j - t0 + n, :]
                raise AssertionError(f"tile {j}+{n} spans chunks")

            def mm(out_ap, w, rhs, **kw):
                nc.tensor.matmul(out_ap, w, rhs, skip_group_check=True,
                                 start=kw.pop("start", False),
                                 stop=kw.pop("stop", False), **kw)

            # Diagonal (mixed) tiles, DoubleRow-paired on their shared spans.
            # Tile j's L span is [64j, 512) (its own mixed bucket holds
            # host-pre-masked lower data), U span is [0, 64j); the mixed
            # bucket's upper part comes from the diagu strips.
            for j in range(0, TPC, 2):
                a, b = BCAP * j, BCAP * (j + 1)
                rhs2 = chunk_rhs(j, 2)
                # shared L span of the pair
                mm(acc[:, b:CAP], wv(j, 2), rhs2[:, :, b:CAP],
                   start=(j == 0), perf_mode=DR)
                # tile j's extra L strip (its own bucket)
                mm(acc[:, a:b], wv(j), chunk_rhs(j)[:, :, a:b])
                if j > 0:
                    # shared U span of the pair
                    mm(acc[:, 0:a], wv(NT + j, 2), rhs2[:, :, 0:a], perf_mode=DR)
                # tile j+1's extra U strip (tile j's bucket columns)
                mm(acc[:, a:b], wv(NT + j + 1), chunk_rhs(j + 1)[:, :, a:b])
                # upper parts of the mixed buckets themselves
                mm(acc[:, a:b], wv(NT + j), du(j))
                mm(acc[:, b : b + BCAP], wv(NT + j + 1), du(j + 1))
            for j in range(TPC, NT, 2):
                mm(acc[:, :], wv(j, 2), chunk_rhs(j, 2),
                   stop=(j == NT - 2), perf_mode=DR)

            out_sb = singles.tile([M, CAP], mybir.dt.float32)
            half = CAP // 2
            nc.vector.tensor_copy(out_sb[:, half:], acc[:, half:])
            nc.scalar.copy(out_sb[:, 0:half], acc[:, 0:half])
            nc.sync.dma_start(out=stats[:, 0:half], in_=out_sb[:, 0:half])
            nc.scalar.dma_start(out=stats[:, half:], in_=out_sb[:, half:])

    nc.compile()
    return nc


def _split_fp8(v, terms=3):
    """Split f64 vector into `terms` fp8 values summing to ~v (12 mantissa bits)."""
    out = []
    r = np.asarray(v, np.float64)
    for _ in range(terms):
        t = r.astype(FP8)
        out.append(t)
        r = r - t.astype(np.float64)
    return out


def _make_wside(outputs, targets):
    """Per-row weight table [N, 8] fp8."""
    out = np.asarray(outputs, np.float64).reshape(-1)
    pos = (np.asarray(targets).reshape(-1) != 0).astype(np.float64)
    cols = [np.ones(N, FP8), pos.astype(FP8)]
    cols += _split_fp8(pos * out)
    cols += _split_fp8(np.exp(out))
    return np.stack(cols, axis=1).astype(FP8)  # [N, 8]


def _build_wmat(wside, core):
    """Per-core weight variants [128, (64+8)*16] fp8.

    Variant j (j<64): weights for local row tile j (absolute tile (8*core+j)%64).
      j < 8  -> L-only variant (diag tiles; U-only twin stored at 64+j)
      j >= 8 -> single variant, L or U half per the tile's position vs the slab
    """
    w = np.zeros((P, NT + TPC, VW), dtype=FP8)
    for j in range(NT):
        t = (TPC * core + j) % NT
        rows = wside[t * P : (t + 1) * P, :]
        if j < TPC:
            w[:, j, 0:NW] = rows
            w[:, NT + j, NW:M] = rows
        elif j < NT - TPC * core:
            w[:, j, NW:M] = rows  # rows above slab columns -> U
        else:
            w[:, j, 0:NW] = rows  # wrapped rows below slab columns -> L
    return np.ascontiguousarray(w.reshape(P, (NT + TPC) * VW))


def _prepare(outputs, targets, node_adj, idx_node):
    """Build per-core in_maps + combine context (slot->column map, multiplicities,
    host-computed contribution of any bucket-overflow columns)."""
    node_adj = np.asarray(node_adj)
    idx = np.asarray(idx_node).reshape(-1).astype(np.int64)
    ucols, mult = np.unique(idx, return_counts=True)
    wside = _make_wside(outputs, targets)

    in_maps = []
    slot_cols = np.full((NCORES, CAP), -1, np.int64)
    overflow = []
    rows128 = np.arange(P)
    s_idx = np.arange(CAP)
    base = P * (s_idx // BCAP)  # first local row of each slot's mixed block

    for d in range(NCORES):
        lo = SLAB * d
        uc = ucols[(ucols >= lo) & (ucols < lo + SLAB)]
        cols_s = np.full(CAP, -1, np.int64)
        for b in range(TPC):
            blk = uc[(uc - lo) // P == b]
            if len(blk) > BCAP:
                overflow.extend(blk[BCAP:].tolist())
                blk = blk[:BCAP]
            cols_s[BCAP * b : BCAP * b + len(blk)] = blk
        slot_cols[d] = cols_s
        valid = cols_s >= 0

        G = (node_adj[:, np.where(valid, cols_s, 0)] != 0).astype(np.float32)
        G[:, ~valid] = 0.0
        # rotate rows: local row r = absolute row (r + 1024d) mod N
        G = np.concatenate([G[lo:], G[:lo]], axis=0)
        lc = np.where(valid, cols_s - lo, -1)  # local split row (diag) per slot
        G[lc[valid], s_idx[valid]] = 0.0       # zero the diagonal
        block = G[base[None, :] + rows128[:, None], s_idx[None, :]]  # [128, CAP]
        lrow = base[None, :] + rows128[:, None]
        diagL = np.where(lrow < lc[None, :], block, 0.0)
        diagU = np.where(lrow > lc[None, :], block, 0.0)
        G[base[None, :] + rows128[:, None], s_idx[None, :]] = diagL
        # tile-major flat layout: adjf[p, CAP*j + s] = G[128j + p, s], split
        # into one contiguous dram tensor per DMA chunk
        adjf = G.reshape(NT, P, CAP).transpose(1, 0, 2).reshape(P, NT * CAP).astype(FP8)
        im = {
            "wmat": _build_wmat(wside, d),
            "diagu": np.ascontiguousarray(diagU.astype(FP8)),
        }
        t0 = 0
        for ci, nt in enumerate(CHUNK_TILES):
            im[f"adj{ci}"] = np.ascontiguousarray(
                adjf[:, t0 * CAP : (t0 + nt) * CAP]
            )
            t0 += nt
        in_maps.append(im)

    mult_of = np.zeros(N, np.int64)
    mult_of[ucols] = mult
    over_loss = _host_cols_loss(outputs, targets, node_adj, overflow, mult_of)
    ctx = {"slot_cols": slot_cols, "mult_of": mult_of, "over_loss": over_loss}
    return in_maps, ctx


def _host_cols_loss(outputs, targets, node_adj, cols, mult_of):
    """Reference-exact loss contribution of a few columns (bucket overflow only)."""
    if not cols:
        return 0.0
    cols = np.asarray(cols, np.int64)
    out = np.asarray(outputs, np.float64).reshape(-1)
    pos = np.asarray(targets).reshape(-1) != 0
    A = node_adj[:, cols] != 0
    r = np.arange(N)[:, None]
    A = A & (r != cols[None, :])
    total = 0.0
    for mask in (A & (r < cols[None, :]), A & (r > cols[None, :])):
        cnt = mask.sum(axis=0)
        poscnt = (mask & pos[:, None]).sum(axis=0)
        sumexp = (mask * np.exp(out)[:, None]).sum(axis=0)
        poslogit = (mask * (pos * out)[:, None]).sum(axis=0)
        valid = (cnt > 0) & (poscnt == 1)
        contrib = np.where(
            valid,
            (np.log(np.maximum(sumexp, 1e-300)) - poslogit) / np.maximum(cnt, 1),
            0.0,
        )
        total += (contrib * mult_of[cols]).sum()
    return total


def _combine(stats_list, ctx):
    """Per-core stats [16, CAP] f32 -> scalar loss (f64 math)."""

    def side_contrib(x):
        cnt, poscnt = x[0], x[1]
        poslogit = x[2] + x[3] + x[4]
        sumexp = x[5] + x[6] + x[7]
        valid = (cnt > 0.5) & (np.abs(poscnt - 1.0) < 0.25)
        lse = np.log(np.where(valid, np.maximum(sumexp, 1e-300), 1.0))
        return np.where(valid, (lse - poslogit) / np.maximum(cnt, 1.0), 0.0)

    total = ctx["over_loss"]
    for d, s in enumerate(stats_list):
        x = np.asarray(s, np.float64)
        contrib = side_contrib(x[0:NW]) + side_contrib(x[NW:M])
        cols = ctx["slot_cols"][d]
        valid = cols >= 0
        total += (contrib[valid] * ctx["mult_of"][cols[valid]]).sum()
    return np.array(total, dtype=np.float32)


def _ensure_axon_hooks_stub():
    """bass_utils imports antenv.axon_hooks when tracing is requested via
    env; the module is absent on some images. Provide a no-op stub so the
    import never crashes (hook=None -> bass_utils skips tracing)."""
    import sys
    import types

    try:
        import antenv.axon_hooks  # noqa: F401
    except ImportError:
        mod = types.ModuleType("antenv.axon_hooks")
        state = {"hook": None}
        mod.set_axon_ntff_profile_hook = lambda h: state.__setitem__("hook", h)
        mod.get_axon_ntff_profile_hook = lambda: state["hook"]
        sys.modules["antenv.axon_hooks"] = mod


def _device_stats(in_maps):
    _ensure_axon_hooks_stub()
    from concourse.bass_utils import run_bass_kernel_spmd

    if "nc" not in _BASS_CACHE:
        _BASS_CACHE["nc"] = _build_bass()
    last_exc = None
    for attempt in range(4):
        try:
            res = run_bass_kernel_spmd(
                _BASS_CACHE["nc"], in_maps, core_ids=list(range(NCORES))
            )
            return [r["stats"] for r in res.results]
        except Exception as e:  # transient NRT/accelerator hiccups
            last_exc = e
            try:
                # a fresh PJRT client usually recovers a transiently
                # "unrecoverable" accelerator; mirrors a process restart
                import jax
                import jax.extend.backend as _jeb

                jax.clear_caches()
                _jeb.clear_backends()
            except Exception:
                pass
            import time

            time.sleep(2.0 * (attempt + 1))
    raise last_exc


def _sim_stats(in_maps):
    """Numpy emulation of the device kernel (same inputs), for logic validation."""
    outs = []
    for m in in_maps:
        adjf = np.concatenate(
            [m[f"adj{ci}"] for ci in range(len(CHUNK_TILES))], axis=1
        ).astype(np.float32)
        diagu = m["diagu"].astype(np.float32)
        w = m["wmat"].reshape(P, NT + TPC, VW).astype(np.float32)
        acc = np.zeros((M, CAP), np.float32)
        for j in range(NT):
            tile = adjf[:, j * CAP : (j + 1) * CAP]
            if j < TPC:
                c0 = BCAP * j
                acc[:, c0:] += w[:, j, :M].T @ tile[:, c0:]
                acc[:, :c0] += w[:, NT + j, :M].T @ tile[:, :c0]
                acc[:, c0 : c0 + BCAP] += w[:, NT + j, :M].T @ diagu[:, c0 : c0 + BCAP]
            else:
                acc += w[:, j, :M].T @ tile
        outs.append(acc)
    return outs


def kernel(outputs, targets, node_adj, idx_node, _simulate=False):
    in_maps, ctx = _prepare(outputs, targets, node_adj, idx_node)
    stats = _sim_stats(in_maps) if _simulate else _device_stats(in_maps)
    return _combine(stats, ctx)


# revision 37
# speedup vs baseline: 1.1446x; 1.1446x over previous
"""Trainium2 Bass kernel for nn_CELoss_4896262717859 (fp8 DoubleRow, gathered
query columns).

For each query column c = idx_node[k] of a sparse adjacency matrix (diagonal
zeroed), a cross-entropy-style loss over the "lower" (r < c) and "upper"
(r > c) neighbor sets:

    contrib_side(c) = [cnt>0 and poscnt==1] * (log(sum_r m exp(out_r)) - poslogit) / cnt

All per-column quantities are sums  sum_r adj[r,c] * w[r]  for
w in {1, pos, pos*out, exp(out)} -> tensor-engine matvecs with a triangular
(L/U) split, computed ONLY for the distinct idx_node columns (~3218 of 8192),
then combined with multiplicities on the host (O(N+K)).

Sharding: core d handles the distinct query columns falling in column slab
[1024d, 1024(d+1)).  Within a slab, columns are bucketed by the 128-row block
containing their diagonal (the "mixed" block); each of the 8 buckets is padded
to a fixed BCAP=56 slots -> exactly 448 column slots per core, so ONE compiled
program (fixed matmul ranges) serves every core and any input.  Rows are
rotated by 1024d so the mixed blocks always land in local row-tiles 0..7.
Bucket overflow beyond 56 distinct columns (a handful for uniform idx_node)
falls back to an exact host-side computation for the overflowed columns only.

Everything streams as fp8e4 (adjacency 0/1 exact; weights hi/mid/lo split
-> ~12 mantissa bits): 3.7 MB/core vs the 32 MB int32 baseline.  Row-tile
pairs run as DoubleRow matmuls (2 fp8 MACs/cell/cycle), halving PE time; the
diagonal tiles pair their shared L/U spans the same way.  The mixed 128-row
block of each column is pre-masked on the host: its lower part (rows < c)
replaces the block in the main slab (covered by the L matmul), its upper part
goes to a small separate diagu[128,448] operand (one extra 56-wide matmul per
diagonal tile).  No on-device casts or mask multiplies remain.

DMA: the adjacency chunks stream through the single gpsimd SWDGE ring in
need order (per-chunk contiguous dram tensors; small leading chunks so
matmuls start early, small final chunk so the compute tail after the last
arrival is short); weights/diagu load concurrently via the sync/scalar HWDGE
rings.  Concurrent chunk rings measured strictly worse (fair-share packet
round-robin delays in-order arrival); per-core streaming tops out ~280 GB/s
regardless of ring mix, making the stream the critical path.  The psum bank
is copied out in halves (ACT+DVE) and written back through both HWDGE rings.
"""

import numpy as np
import ml_dtypes

N = 8192
K = 4096
NCORES = 8
SLAB = N // NCORES        # 1024 columns per slab
P = 128                   # partition / tile edge
NT = N // P               # 64 row tiles
TPC = SLAB // P           # 8 diagonal (mixed) tiles per core
NW = 8                    # weights per side: {1, pos, pl_h, pl_m, pl_l, e_h, e_m, e_l}
M = 2 * NW                # 16 psum partitions (L half = 0:8, U half = 8:16)
VW = 16                   # weight-variant stride (cols); == M, and 16B for fp8
CAP = 448                 # column slots per core (one psum bank)
BCAP = CAP // TPC         # 56 slots per 128-row bucket
CHUNK_TILES = (2, 2, 4, 8, 8, 12, 12, 12, 4)   # row tiles per DMA chunk
CHUNK_ENGS = ("gpsimd",) * 9                   # issuing ring per chunk

BF16 = ml_dtypes.bfloat16
FP8 = ml_dtypes.float8_e4m3fn

_BASS_CACHE = {}


def _build_bass():
    import concourse.tile as tile
    import concourse.mybir as mybir
    from concourse import bacc

    f8 = mybir.dt.float8e4
    DR = mybir.MatmulPerfMode.DoubleRow
    # Bacc (not raw Bass): its compile() runs generate_event_semaphores,
    # which splits multi-sem waits — TRN2 instructions hold at most one.
    nc = bacc.Bacc("TRN2")
    adjs = [
        nc.dram_tensor(f"adj{ci}", [P, nt * CAP], f8, kind="ExternalInput")
        for ci, nt in enumerate(CHUNK_TILES)
    ]
    wmat = nc.dram_tensor("wmat", [P, (NT + TPC) * VW], f8, kind="ExternalInput")
    diagu = nc.dram_tensor("diagu", [P, CAP], f8, kind="ExternalInput")
    stats = nc.dram_tensor("stats", [M, CAP], mybir.dt.float32, kind="ExternalOutput")

    with tile.TileContext(nc) as tc:
        with (
            tc.tile_pool(name="singles", bufs=1) as singles,
            tc.tile_pool(name="psum", bufs=1, space="PSUM") as psum_pool,
        ):
            # weights/diagu on the two HWDGE rings (needed first; they drain
            # while the chunk stream starts), adjacency chunks on the gpsimd
            # SWDGE ring as one in-order FIFO.
            asb = singles.tile([P, NT + TPC, VW], f8)
            nc.sync.dma_start(out=asb, in_=wmat[:, :])
            dsb = singles.tile([P, CAP], f8)
            nc.scalar.dma_start(out=dsb, in_=diagu[:, :])
            engs = {"sync": nc.sync, "scalar": nc.scalar, "gpsimd": nc.gpsimd}
            chunks = []   # (tile, first_tile, ntiles)
            t0 = 0
            for ci, nt in enumerate(CHUNK_TILES):
                t = singles.tile([P, nt, CAP], f8, name=f"ch{ci}")
                engs[CHUNK_ENGS[ci]].dma_start(out=t, in_=adjs[ci][:, :])
                chunks.append((t, t0, nt))
                t0 += nt
            assert t0 == NT

            acc = psum_pool.tile([M, CAP], mybir.dt.float32, name="acc")

            def wv(v, n=1):
                return asb[:, v : v + n, :]

            def du(b):  # diagu strip for mixed block b
                return dsb[:, BCAP * b : BCAP * (b + 1)]

            def chunk_rhs(j, n=1):
                for t, t0, nt in chunks:
                    if t0 <= j and j + n <= t0 + nt:
                        return t[:, j - t0 : j - t0 + n, :]
                raise AssertionError(f"tile {j}+{n} spans chunks")

            def mm(out_ap, w, rhs, **kw):
                nc.tensor.matmul(out_ap, w, rhs, skip_group_check=True,
                                 start=kw.pop("start", False),
                                 stop=kw.pop("stop", False), **kw)

            # Diagonal (mixed) tiles, DoubleRow-paired on their shared spans.
            # Tile j's L span is [64j, 512) (its own mixed bucket holds
            # host-pre-masked lower data), U span is [0, 64j); the mixed
            # bucket's upper part comes from the diagu strips.
            for j in range(0, TPC, 2):
                a, b = BCAP * j, BCAP * (j + 1)
                rhs2 = chunk_rhs(j, 2)
                # shared L span of the pair
                mm(acc[:, b:CAP], wv(j, 2), rhs2[:, :, b:CAP],
                   start=(j == 0), perf_mode=DR)
                # tile j's extra L strip (its own bucket)
                mm(acc[:, a:b], wv(j), chunk_rhs(j)[:, :, a:b])
                if j > 0:
                    # shared U span of the pair
                    mm(acc[:, 0:a], wv(NT + j, 2), rhs2[:, :, 0:a], perf_mode=DR)
                # tile j+1's extra U strip (tile j's bucket columns)
                mm(acc[:, a:b], wv(NT + j + 1), chunk_rhs(j + 1)[:, :, a:b])
                # upper parts of the mixed buckets themselves
                mm(acc[:, a:b], wv(NT + j), du(j))
                mm(acc[:, b : b + BCAP], wv(NT + j + 1), du(j + 1))
            for j in range(TPC, NT, 2):
                mm(acc[:, :], wv(j, 2), chunk_rhs(j, 2),
                   stop=(j == NT - 2), perf_mode=DR)

            out_sb = singles.tile([M, CAP], mybir.dt.float32)
            half = CAP // 2
            nc.vector.tensor_copy(out_sb[:, half:], acc[:, half:])
            nc.scalar.copy(out_sb[:, 0:half], acc[:, 0:half])
            nc.sync.dma_start(out=stats[:, 0:half], in_=out_sb[:, 0:half])
            nc.scalar.dma_start(out=stats[:, half:], in_=out_sb[:, half:])

    nc.compile()
    return nc


def _split_fp8(v, terms=3):
    """Split f64 vector into `terms` fp8 values summing to ~v (12 mantissa bits)."""
    out = []
    r = np.asarray(v, np.float64)
    for _ in range(terms):
        t = r.astype(FP8)
        out.append(t)
        r = r - t.astype(np.float64)
    return out


def _make_wside(outputs, targets):
    """Per-row weight table [N, 8] fp8."""
    out = np.asarray(outputs, np.float64).reshape(-1)
    pos = (np.asarray(targets).reshape(-1) != 0).astype(np.float64)
    cols = [np.ones(N, FP8), pos.astype(FP8)]
    cols += _split_fp8(pos * out)
    cols += _split_fp8(np.exp(out))
    return np.stack(cols, axis=1).astype(FP8)  # [N, 8]


def _build_wmat(wside, core):
    """Per-core weight variants [128, (64+8)*16] fp8.

    Variant j (j<64): weights for local row tile j (absolute tile (8*core+j)%64).
      j < 8  -> L-only variant (diag tiles; U-only twin stored at 64+j)
      j >= 8 -> single variant, L or U half per the tile's position vs the slab
    """
    w = np.zeros((P, NT + TPC, VW), dtype=FP8)
    for j in range(NT):
        t = (TPC * core + j) % NT
        rows = wside[t * P : (t + 1) * P, :]
        if j < TPC:
            w[:, j, 0:NW] = rows
            w[:, NT + j, NW:M] = rows
        elif j < NT - TPC * core:
            w[:, j, NW:M] = rows  # rows above slab columns -> U
        else:
            w[:, j, 0:NW] = rows  # wrapped rows below slab columns -> L
    return np.ascontiguousarray(w.reshape(P, (NT + TPC) * VW))


def _prepare(outputs, targets, node_adj, idx_node):
    """Build per-core in_maps + combine context (slot->column map, multiplicities,
    host-computed contribution of any bucket-overflow columns)."""
    node_adj = np.asarray(node_adj)
    idx = np.asarray(idx_node).reshape(-1).astype(np.int64)
    ucols, mult = np.unique(idx, return_counts=True)
    wside = _make_wside(outputs, targets)

    in_maps = []
    slot_cols = np.full((NCORES, CAP), -1, np.int64)
    overflow = []
    rows128 = np.arange(P)
    s_idx = np.arange(CAP)
    base = P * (s_idx // BCAP)  # first local row of each slot's mixed block

    for d in range(NCORES):
        lo = SLAB * d
        uc = ucols[(ucols >= lo) & (ucols < lo + SLAB)]
        cols_s = np.full(CAP, -1, np.int64)
        for b in range(TPC):
            blk = uc[(uc - lo) // P == b]
            if len(blk) > BCAP:
                overflow.extend(blk[BCAP:].tolist())
                blk = blk[:BCAP]
            cols_s[BCAP * b : BCAP * b + len(blk)] = blk
        slot_cols[d] = cols_s
        valid = cols_s >= 0

        G = (node_adj[:, np.where(valid, cols_s, 0)] != 0).astype(np.float32)
        G[:, ~valid] = 0.0
        # rotate rows: local row r = absolute row (r + 1024d) mod N
        G = np.concatenate([G[lo:], G[:lo]], axis=0)
        lc = np.where(valid, cols_s - lo, -1)  # local split row (diag) per slot
        G[lc[valid], s_idx[valid]] = 0.0       # zero the diagonal
        block = G[base[None, :] + rows128[:, None], s_idx[None, :]]  # [128, CAP]
        lrow = base[None, :] + rows128[:, None]
        diagL = np.where(lrow < lc[None, :], block, 0.0)
        diagU = np.where(lrow > lc[None, :], block, 0.0)
        G[base[None, :] + rows128[:, None], s_idx[None, :]] = diagL
        # tile-major flat layout: adjf[p, CAP*j + s] = G[128j + p, s], split
        # into one contiguous dram tensor per DMA chunk
        adjf = G.reshape(NT, P, CAP).transpose(1, 0, 2).reshape(P, NT * CAP).astype(FP8)
        im = {
            "wmat": _build_wmat(wside, d),
            "diagu": np.ascontiguousarray(diagU.astype(FP8)),
        }
        t0 = 0
        for ci, nt in enumerate(CHUNK_TILES):
            im[f"adj{ci}"] = np.ascontiguousarray(
                adjf[:, t0 * CAP : (t0 + nt) * CAP]
            )
            t0 += nt
        in_maps.append(im)

    mult_of = np.zeros(N, np.int64)
    mult_of[ucols] = mult
    over_loss = _host_cols_loss(outputs, targets, node_adj, overflow, mult_of)
    ctx = {"slot_cols": slot_cols, "mult_of": mult_of, "over_loss": over_loss}
    return in_maps, ctx


def _host_cols_loss(outputs, targets, node_adj, cols, mult_of):
    """Reference-exact loss contribution of a few columns (bucket overflow only)."""
    if not cols:
        return 0.0
    cols = np.asarray(cols, np.int64)
    out = np.asarray(outputs, np.float64).reshape(-1)
    pos = np.asarray(targets).reshape(-1) != 0
    A = node_adj[:, cols] != 0
    r = np.arange(N)[:, None]
    A = A & (r != cols[None, :])
    total = 0.0
    for mask in (A & (r < cols[None, :]), A & (r > cols[None, :])):
        cnt = mask.sum(axis=0)
        poscnt = (mask & pos[:, None]).sum(axis=0)
        sumexp = (mask * np.exp(out)[:, None]).sum(axis=0)
        poslogit = (mask * (pos * out)[:, None]).sum(axis=0)
        valid = (cnt > 0) & (poscnt == 1)
        contrib = np.where(
            valid,
            (np.log(np.maximum(sumexp, 1e-300)) - poslogit) / np.maximum(cnt, 1),
            0.0,
        )
        total += (contrib * mult_of[cols]).sum()
    return total


def _combine(stats_list, ctx):
    """Per-core stats [16, CAP] f32 -> scalar loss (f64 math)."""

    def side_contrib(x):
        cnt, poscnt = x[0], x[1]
        poslogit = x[2] + x[3] + x[4]
        sumexp = x[5] + x[6] + x[7]
        valid = (cnt > 0.5) & (np.abs(poscnt - 1.0) < 0.25)
        lse = np.log(np.where(valid, np.maximum(sumexp, 1e-300), 1.0))
        return np.where(valid, (lse - poslogit) / np.maximum(cnt, 1.0), 0.0)

    total = ctx["over_loss"]
    for d, s in enumerate(stats_list):
        x = np.asarray(s, np.float64)
        contrib = side_contrib(x[0:NW]) + side_contrib(x[NW:M])
        cols = ctx["slot_cols"][d]
        valid = cols >= 0
        total += (contrib[valid] * ctx["mult_of"][cols[valid]]).sum()
    return np.array(total, dtype=np.float32)


def _ensure_axon_hooks_stub():
    """bass_utils imports antenv.axon_hooks when tracing is requested via
    env; the module is absent on some images. Provide a no-op stub so the
    import never crashes (hook=None -> bass_utils skips tracing)."""
    import sys
    import types

    try:
        import antenv.axon_hooks  # noqa: F401
    except ImportError:
        mod = types.ModuleType("antenv.axon_hooks")
        state = {"hook": None}
        mod.set_axon_ntff_profile_hook = lambda h: state.__setitem__("hook", h)
        mod.get_axon_ntff_profile_hook = lambda: state["hook"]
        sys.modules["antenv.axon_hooks"] = mod


def _device_stats(in_maps):
    _ensure_axon_hooks_stub()
    from concourse.bass_utils import run_bass_kernel_spmd

    if "nc" not in _BASS_CACHE:
        _BASS_CACHE["nc"] = _build_bass()
    last_exc = None
    for attempt in range(4):
        try:
            res = run_bass_kernel_spmd(
                _BASS_CACHE["nc"], in_maps, core_ids=list(range(NCORES))
            )
            return [r["stats"] for r in res.results]
        except Exception as e:  # transient NRT/accelerator hiccups
            last_exc = e
            try:
                # a fresh PJRT client usually recovers a transiently
                # "unrecoverable" accelerator; mirrors a process restart
                import jax
                import jax.extend.backend as _jeb

                jax.clear_caches()
                _jeb.clear_backends()
            except Exception:
                pass
            import time

            time.sleep(2.0 * (attempt + 1))
    raise last_exc


def _sim_stats(in_maps):
    """Numpy emulation of the device kernel (same inputs), for logic validation."""
    outs = []
    for m in in_maps:
        adjf = np.concatenate(
            [m[f"adj{ci}"] for ci in range(len(CHUNK_TILES))], axis=1
        ).astype(np.float32)
        diagu = m["diagu"].astype(np.float32)
        w = m["wmat"].reshape(P, NT + TPC, VW).astype(np.float32)
        acc = np.zeros((M, CAP), np.float32)
        for j in range(NT):
            tile = adjf[:, j * CAP : (j + 1) * CAP]
            if j < TPC:
                c0 = BCAP * j
                acc[:, c0:] += w[:, j, :M].T @ tile[:, c0:]
                acc[:, :c0] += w[:, NT + j, :M].T @ tile[:, :c0]
                acc[:, c0 : c0 + BCAP] += w[:, NT + j, :M].T @ diagu[:, c0 : c0 + BCAP]
            else:
                acc += w[:, j, :M].T @ tile
        outs.append(acc)
    return outs


def kernel(outputs, targets, node_adj, idx_node, _simulate=False):
    in_maps, ctx = _prepare(outputs, targets, node_adj, idx_node)
    stats = _sim_stats(in_maps) if _simulate else _device_stats(in_maps)
    return _combine(stats, ctx)


# revision 38
# speedup vs baseline: 1.1602x; 1.0136x over previous
"""Trainium2 Bass kernel for nn_CELoss_4896262717859 (fp8 DoubleRow, gathered
query columns).

For each query column c = idx_node[k] of a sparse adjacency matrix (diagonal
zeroed), a cross-entropy-style loss over the "lower" (r < c) and "upper"
(r > c) neighbor sets:

    contrib_side(c) = [cnt>0 and poscnt==1] * (log(sum_r m exp(out_r)) - poslogit) / cnt

All per-column quantities are sums  sum_r adj[r,c] * w[r]  for
w in {1, pos, pos*out, exp(out)} -> tensor-engine matvecs with a triangular
(L/U) split, computed ONLY for the distinct idx_node columns (~3218 of 8192),
then combined with multiplicities on the host (O(N+K)).

Sharding: core d handles the distinct query columns falling in column slab
[1024d, 1024(d+1)).  Within a slab, columns are bucketed by the 128-row block
containing their diagonal (the "mixed" block); each of the 8 buckets is padded
to a fixed BCAP=56 slots -> exactly 448 column slots per core, so ONE compiled
program (fixed matmul ranges) serves every core and any input.  Rows are
rotated by 1024d so the mixed blocks always land in local row-tiles 0..7.
Bucket overflow beyond 56 distinct columns (a handful for uniform idx_node)
falls back to an exact host-side computation for the overflowed columns only.

Everything streams as fp8e4 (adjacency 0/1 exact; weights hi/mid/lo split
-> ~12 mantissa bits): 3.7 MB/core vs the 32 MB int32 baseline.  Row-tile
pairs run as DoubleRow matmuls (2 fp8 MACs/cell/cycle), halving PE time; the
diagonal tiles pair their shared L/U spans the same way.  The mixed 128-row
block of each column is pre-masked on the host: its lower part (rows < c)
replaces the block in the main slab (covered by the L matmul), its upper part
goes to a small separate diagu[128,448] operand (one extra 56-wide matmul per
diagonal tile).  No on-device casts or mask multiplies remain.

DMA: the adjacency chunks stream through the single gpsimd SWDGE ring in
need order (per-chunk contiguous dram tensors; small leading chunks so
matmuls start early, small final chunk so the compute tail after the last
arrival is short); weights/diagu load concurrently via the sync/scalar HWDGE
rings.  Concurrent chunk rings measured strictly worse (fair-share packet
round-robin delays in-order arrival); per-core streaming tops out ~280 GB/s
regardless of ring mix, making the stream the critical path.  The psum bank
is copied out in halves (ACT+DVE) and written back through both HWDGE rings.
"""

import numpy as np
import ml_dtypes

N = 8192
K = 4096
NCORES = 8
SLAB = N // NCORES        # 1024 columns per slab
P = 128                   # partition / tile edge
NT = N // P               # 64 row tiles
TPC = SLAB // P           # 8 diagonal (mixed) tiles per core
NW = 8                    # weights per side: {1, pos, pl_h, pl_m, pl_l, e_h, e_m, e_l}
M = 2 * NW                # 16 psum partitions (L half = 0:8, U half = 8:16)
VW = 16                   # weight-variant stride (cols); == M, and 16B for fp8
CAP = 416                 # column slots per core (one psum bank)
BCAP = CAP // TPC         # 52 slots per 128-row bucket
CHUNK_TILES = (2, 2, 4, 8, 8, 12, 12, 12, 4)   # row tiles per DMA chunk
CHUNK_ENGS = ("gpsimd",) * 9                   # issuing ring per chunk

BF16 = ml_dtypes.bfloat16
FP8 = ml_dtypes.float8_e4m3fn

_BASS_CACHE = {}


def _build_bass():
    import concourse.tile as tile
    import concourse.mybir as mybir
    from concourse import bacc

    f8 = mybir.dt.float8e4
    DR = mybir.MatmulPerfMode.DoubleRow
    # Bacc (not raw Bass): its compile() runs generate_event_semaphores,
    # which splits multi-sem waits — TRN2 instructions hold at most one.
    nc = bacc.Bacc("TRN2")
    adjs = [
        nc.dram_tensor(f"adj{ci}", [P, nt * CAP], f8, kind="ExternalInput")
        for ci, nt in enumerate(CHUNK_TILES)
    ]
    wmat = nc.dram_tensor("wmat", [P, (NT + TPC) * VW], f8, kind="ExternalInput")
    diagu = nc.dram_tensor("diagu", [P, CAP], f8, kind="ExternalInput")
    stats = nc.dram_tensor("stats", [M, CAP], mybir.dt.float32, kind="ExternalOutput")

    with tile.TileContext(nc) as tc:
        with (
            tc.tile_pool(name="singles", bufs=1) as singles,
            tc.tile_pool(name="psum", bufs=1, space="PSUM") as psum_pool,
        ):
            # weights/diagu on the two HWDGE rings (needed first; they drain
            # while the chunk stream starts), adjacency chunks on the gpsimd
            # SWDGE ring as one in-order FIFO.
            asb = singles.tile([P, NT + TPC, VW], f8)
            nc.sync.dma_start(out=asb, in_=wmat[:, :])
            dsb = singles.tile([P, CAP], f8)
            nc.scalar.dma_start(out=dsb, in_=diagu[:, :])
            engs = {"sync": nc.sync, "scalar": nc.scalar, "gpsimd": nc.gpsimd}
            chunks = []   # (tile, first_tile, ntiles)
            t0 = 0
            for ci, nt in enumerate(CHUNK_TILES):
                t = singles.tile([P, nt, CAP], f8, name=f"ch{ci}")
                engs[CHUNK_ENGS[ci]].dma_start(out=t, in_=adjs[ci][:, :])
                chunks.append((t, t0, nt))
                t0 += nt
            assert t0 == NT

            acc = psum_pool.tile([M, CAP], mybir.dt.float32, name="acc")

            def wv(v, n=1):
                return asb[:, v : v + n, :]

            def du(b):  # diagu strip for mixed block b
                return dsb[:, BCAP * b : BCAP * (b + 1)]

            def chunk_rhs(j, n=1):
                for t, t0, nt in chunks:
                    if t0 <= j and j + n <= t0 + nt:
                        return t[:, j - t0 : j - t0 + n, :]
                raise AssertionError(f"tile {j}+{n} spans chunks")

            def mm(out_ap, w, rhs, **kw):
                nc.tensor.matmul(out_ap, w, rhs, skip_group_check=True,
                                 start=kw.pop("start", False),
                                 stop=kw.pop("stop", False), **kw)

            # Diagonal (mixed) tiles, DoubleRow-paired on their shared spans.
            # Tile j's L span is [64j, 512) (its own mixed bucket holds
            # host-pre-masked lower data), U span is [0, 64j); the mixed
            # bucket's upper part comes from the diagu strips.
            for j in range(0, TPC, 2):
                a, b = BCAP * j, BCAP * (j + 1)
                rhs2 = chunk_rhs(j, 2)
                # shared L span of the pair
                mm(acc[:, b:CAP], wv(j, 2), rhs2[:, :, b:CAP],
                   start=(j == 0), perf_mode=DR)
                # tile j's extra L strip (its own bucket)
                mm(acc[:, a:b], wv(j), chunk_rhs(j)[:, :, a:b])
                if j > 0:
                    # shared U span of the pair
                    mm(acc[:, 0:a], wv(NT + j, 2), rhs2[:, :, 0:a], perf_mode=DR)
                # tile j+1's extra U strip (tile j's bucket columns)
                mm(acc[:, a:b], wv(NT + j + 1), chunk_rhs(j + 1)[:, :, a:b])
                # upper parts of the mixed buckets themselves
                mm(acc[:, a:b], wv(NT + j), du(j))
                mm(acc[:, b : b + BCAP], wv(NT + j + 1), du(j + 1))
            for j in range(TPC, NT, 2):
                mm(acc[:, :], wv(j, 2), chunk_rhs(j, 2),
                   stop=(j == NT - 2), perf_mode=DR)

            out_sb = singles.tile([M, CAP], mybir.dt.float32)
            half = CAP // 2
            nc.vector.tensor_copy(out_sb[:, half:], acc[:, half:])
            nc.scalar.copy(out_sb[:, 0:half], acc[:, 0:half])
            nc.sync.dma_start(out=stats[:, 0:half], in_=out_sb[:, 0:half])
            nc.scalar.dma_start(out=stats[:, half:], in_=out_sb[:, half:])

    nc.compile()
    return nc


def _split_fp8(v, terms=3):
    """Split f64 vector into `terms` fp8 values summing to ~v (12 mantissa bits)."""
    out = []
    r = np.asarray(v, np.float64)
    for _ in range(terms):
        t = r.astype(FP8)
        out.append(t)
        r = r - t.astype(np.float64)
    return out


def _make_wside(outputs, targets):
    """Per-row weight table [N, 8] fp8."""
    out = np.asarray(outputs, np.float64).reshape(-1)
    pos = (np.asarray(targets).reshape(-1) != 0).astype(np.float64)
    cols = [np.ones(N, FP8), pos.astype(FP8)]
    cols += _split_fp8(pos * out)
    cols += _split_fp8(np.exp(out))
    return np.stack(cols, axis=1).astype(FP8)  # [N, 8]


def _build_wmat(wside, core):
    """Per-core weight variants [128, (64+8)*16] fp8.

    Variant j (j<64): weights for local row tile j (absolute tile (8*core+j)%64).
      j < 8  -> L-only variant (diag tiles; U-only twin stored at 64+j)
      j >= 8 -> single variant, L or U half per the tile's position vs the slab
    """
    w = np.zeros((P, NT + TPC, VW), dtype=FP8)
    for j in range(NT):
        t = (TPC * core + j) % NT
        rows = wside[t * P : (t + 1) * P, :]
        if j < TPC:
            w[:, j, 0:NW] = rows
            w[:, NT + j, NW:M] = rows
        elif j < NT - TPC * core:
            w[:, j, NW:M] = rows  # rows above slab columns -> U
        else:
            w[:, j, 0:NW] = rows  # wrapped rows below slab columns -> L
    return np.ascontiguousarray(w.reshape(P, (NT + TPC) * VW))


def _prepare(outputs, targets, node_adj, idx_node):
    """Build per-core in_maps + combine context (slot->column map, multiplicities,
    host-computed contribution of any bucket-overflow columns)."""
    node_adj = np.asarray(node_adj)
    idx = np.asarray(idx_node).reshape(-1).astype(np.int64)
    ucols, mult = np.unique(idx, return_counts=True)
    wside = _make_wside(outputs, targets)

    in_maps = []
    slot_cols = np.full((NCORES, CAP), -1, np.int64)
    overflow = []
    rows128 = np.arange(P)
    s_idx = np.arange(CAP)
    base = P * (s_idx // BCAP)  # first local row of each slot's mixed block

    for d in range(NCORES):
        lo = SLAB * d
        uc = ucols[(ucols >= lo) & (ucols < lo + SLAB)]
        cols_s = np.full(CAP, -1, np.int64)
        for b in range(TPC):
            blk = uc[(uc - lo) // P == b]
            if len(blk) > BCAP:
                overflow.extend(blk[BCAP:].tolist())
                blk = blk[:BCAP]
            cols_s[BCAP * b : BCAP * b + len(blk)] = blk
        slot_cols[d] = cols_s
        valid = cols_s >= 0

        G = (node_adj[:, np.where(valid, cols_s, 0)] != 0).astype(np.float32)
        G[:, ~valid] = 0.0
        # rotate rows: local row r = absolute row (r + 1024d) mod N
        G = np.concatenate([G[lo:], G[:lo]], axis=0)
        lc = np.where(valid, cols_s - lo, -1)  # local split row (diag) per slot
        G[lc[valid], s_idx[valid]] = 0.0       # zero the diagonal
        block = G[base[None, :] + rows128[:, None], s_idx[None, :]]  # [128, CAP]
        lrow = base[None, :] + rows128[:, None]
        diagL = np.where(lrow < lc[None, :], block, 0.0)
        diagU = np.where(lrow > lc[None, :], block, 0.0)
        G[base[None, :] + rows128[:, None], s_idx[None, :]] = diagL
        # tile-major flat layout: adjf[p, CAP*j + s] = G[128j + p, s], split
        # into one contiguous dram tensor per DMA chunk
        adjf = G.reshape(NT, P, CAP).transpose(1, 0, 2).reshape(P, NT * CAP).astype(FP8)
        im = {
            "wmat": _build_wmat(wside, d),
            "diagu": np.ascontiguousarray(diagU.astype(FP8)),
        }
        t0 = 0
        for ci, nt in enumerate(CHUNK_TILES):
            im[f"adj{ci}"] = np.ascontiguousarray(
                adjf[:, t0 * CAP : (t0 + nt) * CAP]
            )
            t0 += nt
        in_maps.append(im)

    mult_of = np.zeros(N, np.int64)
    mult_of[ucols] = mult
    over_loss = _host_cols_loss(outputs, targets, node_adj, overflow, mult_of)
    ctx = {"slot_cols": slot_cols, "mult_of": mult_of, "over_loss": over_loss}
    return in_maps, ctx


def _host_cols_loss(outputs, targets, node_adj, cols, mult_of):
    """Reference-exact loss contribution of a few columns (bucket overflow only)."""
    if not cols:
        return 0.0
    cols = np.asarray(cols, np.int64)
    out = np.asarray(outputs, np.float64).reshape(-1)
    pos = np.asarray(targets).reshape(-1) != 0
    A = node_adj[:, cols] != 0
    r = np.arange(N)[:, None]
    A = A & (r != cols[None, :])
    total = 0.0
    for mask in (A & (r < cols[None, :]), A & (r > cols[None, :])):
        cnt = mask.sum(axis=0)
        poscnt = (mask & pos[:, None]).sum(axis=0)
        sumexp = (mask * np.exp(out)[:, None]).sum(axis=0)
        poslogit = (mask * (pos * out)[:, None]).sum(axis=0)
        valid = (cnt > 0) & (poscnt == 1)
        contrib = np.where(
            valid,
            (np.log(np.maximum(sumexp, 1e-300)) - poslogit) / np.maximum(cnt, 1),
            0.0,
        )
        total += (contrib * mult_of[cols]).sum()
    return total


def _combine(stats_list, ctx):
    """Per-core stats [16, CAP] f32 -> scalar loss (f64 math)."""

    def side_contrib(x):
        cnt, poscnt = x[0], x[1]
        poslogit = x[2] + x[3] + x[4]
        sumexp = x[5] + x[6] + x[7]
        valid = (cnt > 0.5) & (np.abs(poscnt - 1.0) < 0.25)
        lse = np.log(np.where(valid, np.maximum(sumexp, 1e-300), 1.0))
        return np.where(valid, (lse - poslogit) / np.maximum(cnt, 1.0), 0.0)

    total = ctx["over_loss"]
    for d, s in enumerate(stats_list):
        x = np.asarray(s, np.float64)
        contrib = side_contrib(x[0:NW]) + side_contrib(x[NW:M])
        cols = ctx["slot_cols"][d]
        valid = cols >= 0
        total += (contrib[valid] * ctx["mult_of"][cols[valid]]).sum()
    return np.array(total, dtype=np.float32)


def _ensure_axon_hooks_stub():
    """bass_utils imports antenv.axon_hooks when tracing is requested via
    env; the module is absent on some images. Provide a no-op stub so the
    import never crashes (hook=None -> bass_utils skips tracing)."""
    import sys
    import types

    try:
        import antenv.axon_hooks  # noqa: F401
    except ImportError:
        mod = types.ModuleType("antenv.axon_hooks")
        state = {"hook": None}
        mod.set_axon_ntff_profile_hook = lambda h: state.__setitem__("hook", h)
        mod.get_axon_ntff_profile_hook = lambda: state["hook"]
        sys.modules["antenv.axon_hooks"] = mod


def _device_stats(in_maps):
    _ensure_axon_hooks_stub()
    from concourse.bass_utils import run_bass_kernel_spmd

    if "nc" not in _BASS_CACHE:
        _BASS_CACHE["nc"] = _build_bass()
    last_exc = None
    for attempt in range(4):
        try:
            res = run_bass_kernel_spmd(
                _BASS_CACHE["nc"], in_maps, core_ids=list(range(NCORES))
            )
            return [r["stats"] for r in res.results]
        except Exception as e:  # transient NRT/accelerator hiccups
            last_exc = e
            try:
                # a fresh PJRT client usually recovers a transiently
                # "unrecoverable" accelerator; mirrors a process restart
                import jax
                import jax.extend.backend as _jeb

                jax.clear_caches()
                _jeb.clear_backends()
            except Exception:
                pass
            import time

            time.sleep(2.0 * (attempt + 1))
    raise last_exc


def _sim_stats(in_maps):
    """Numpy emulation of the device kernel (same inputs), for logic validation."""
    outs = []
    for m in in_maps:
        adjf = np.concatenate(
            [m[f"adj{ci}"] for ci in range(len(CHUNK_TILES))], axis=1
        ).astype(np.float32)
        diagu = m["diagu"].astype(np.float32)
        w = m["wmat"].reshape(P, NT + TPC, VW).astype(np.float32)
        acc = np.zeros((M, CAP), np.float32)
        for j in range(NT):
            tile = adjf[:, j * CAP : (j + 1) * CAP]
            if j < TPC:
                c0 = BCAP * j
                acc[:, c0:] += w[:, j, :M].T @ tile[:, c0:]
                acc[:, :c0] += w[:, NT + j, :M].T @ tile[:, :c0]
                acc[:, c0 : c0 + BCAP] += w[:, NT + j, :M].T @ diagu[:, c0 : c0 + BCAP]
            else:
                acc += w[:, j, :M].T @ tile
        outs.append(acc)
    return outs


def kernel(outputs, targets, node_adj, idx_node, _simulate=False):
    in_maps, ctx = _prepare(outputs, targets, node_adj, idx_node)
    stats = _sim_stats(in_maps) if _simulate else _device_stats(in_maps)
    return _combine(stats, ctx)
